# revision 5
# baseline (speedup 1.0000x reference)
"""Multi-head attention (B=8, N=1024, DIM=768, H=12) on 8 Trainium2 NeuronCores.

Sharding: pure data-parallel over the batch dimension — core c computes batch
element c end-to-end (qkv projection, softmax attention, output projection).
No collectives needed.

Numerics: matmul inputs in bf16 (x, weights, q/k, v, exp(P)) with fp32 PSUM
accumulation; softmax denominator, reciprocal, normalization and bias in
fp32.

Schedule (v3): heads are processed in PAIRS (2t, 2t+1), j-synchronized, so
the two K=64 QK^T matmuls land on disjoint row-groups of the PE array
(head 2t lives on partitions 0-63, head 2t+1 on 64-127 of the qk pair
tile). Issued back-to-back they execute CONCURRENTLY (row tiling via
auto-derived tile_position (0,0)/(64,0)), halving QK^T time - the single
largest PE consumer in the v2 trace (53.8us of 192us PE-busy).

  per pair t, per j: exp_a(j), exp_b(j) [ACT]; ST-pair(j+1) (concurrent);
  PV_a(j) into OT_a. ex_b tiles are buffered in SBUF; PV_b for all j drains
  at the pair boundary (after rchain_a frees OT banks) so PSUM stays within
  8 banks: st(2x2) + ot(3) + s1(1) + lbc borrows an st slot.

  stage-1 (qkv projection) for pair t+1 runs as PE filler inside pair t's
  loop; output projection partials run as filler inside pair 5; only the
  thin k=5 proj pass remains at the end.

DMA: inputs split across both HWDGE queues (sync+scalar) with xT halves in
parallel so the first matmul starts ~8us earlier; y output rows alternate
queues.
"""

import os
import sys

for _p in ("/opt/trn_rl_repo",):
    if _p not in sys.path:
        sys.path.insert(0, _p)

import ml_dtypes
import numpy as np

import concourse.bass as bass
import concourse.tile as tile
from concourse import bacc, mybir

B, N, DIM, H = 8, 1024, 768, 12
D = DIM // H  # 64
SCALE = D ** -0.5
P = 128
KT = DIM // P        # 6 contraction tiles over dim
NT = N // P          # 8 tiles over sequence
NPAIR = H // 2       # 6 head pairs
FP = mybir.dt.float32
BF = mybir.dt.bfloat16
MMDT = BF
NP_MMDT = ml_dtypes.bfloat16


def _chunks(total, size):
    return [(lo, min(lo + size, total)) for lo in range(0, total, size)]


def build_nc():
    nc = bacc.Bacc(None, target_bir_lowering=False)
    xT = nc.dram_tensor("xT", [DIM, N], MMDT, kind="ExternalInput")
    # wqkT columns are pair-blocked: [q_t | k_t] of 128 cols each, t=0..5
    wqkT = nc.dram_tensor("wqkT", [DIM, 2 * DIM], MMDT, kind="ExternalInput")
    wvT = nc.dram_tensor("wvT", [DIM, DIM], MMDT, kind="ExternalInput")
    wpT = nc.dram_tensor("wpT", [DIM, DIM], MMDT, kind="ExternalInput")
    bias = nc.dram_tensor("bias", [1, DIM], FP, kind="ExternalInput")
    y = nc.dram_tensor("y", [N, DIM], FP, kind="ExternalOutput")

    with tile.TileContext(nc) as tc:
        with nc.allow_low_precision(reason="bf16 matmul inputs"):
            _body(tc, xT, wqkT, wvT, wpT, bias, y)
    nc.compile()
    return nc


def _body(tc, xT, wqkT, wvT, wpT, bias, y):
    nc = tc.nc
    Exp = mybir.ActivationFunctionType.Exp
    Mult = mybir.AluOpType.mult
    Add = mybir.AluOpType.add

    from contextlib import ExitStack
    with tc.tile_pool(name="persist", bufs=1) as persist:
      with ExitStack() as s12:
        s1w = s12.enter_context(tc.tile_pool(name="s1w", bufs=1))
        expa_p = s12.enter_context(tc.tile_pool(name="expa", bufs=3))
        expb_p = s12.enter_context(tc.tile_pool(name="expb", bufs=9))
        rp = s12.enter_context(tc.tile_pool(name="rp", bufs=2))
        s1ps = s12.enter_context(tc.tile_pool(name="s1ps", bufs=1, space="PSUM"))
        stps = s12.enter_context(tc.tile_pool(name="stps", bufs=2, space="PSUM"))
        otps = s12.enter_context(tc.tile_pool(name="otps", bufs=3, space="PSUM"))

        # qkT_sb tile index 2t = q of pair t, 2t+1 = k of pair t; rows (h%2,d)
        qkT_sb = persist.tile([P, 2 * KT, N], MMDT)     # 24 KB/part
        v_sb = persist.tile([P, NT, H, D + 1], MMDT)    # 12.7 KB/part
        oT_sb = persist.tile([P, KT, N], MMDT)          # 12 KB/part
        bias_sb = persist.tile([P, DIM], FP)            # 3 KB/part
        y_acc = persist.tile([P, NT, DIM], FP)          # 24 KB/part
        ones_f32r = persist.tile([1, P], mybir.dt.float32r)
        ones_stg = persist.tile([1, P], FP)
        nc.sync.dma_start(out=bias_sb, in_=bias[:].to_broadcast((P, DIM)))
        nc.vector.memset(v_sb[:, :, :, D], 1.0)
        nc.vector.memset(ones_stg, 1.0)
        nc.vector.tensor_copy(out=ones_f32r, in_=ones_stg)

        xT_sb = s1w.tile([P, KT, N], MMDT)              # 12 KB/part
        wqkT_sb = s1w.tile([P, KT, 2 * DIM], MMDT)      # 18 KB/part
        wvT_sb = s1w.tile([P, KT, DIM], MMDT)           # 9 KB/part
        wpT_sb = s1w.tile([P, KT, DIM], MMDT)           # 9 KB/part

        xTr = xT[:].rearrange("(t p) n -> t p n", p=P)
        wqkr = wqkT[:].rearrange("(t p) m -> t p m", p=P)
        wvr = wvT[:].rearrange("(t p) m -> t p m", p=P)
        wpr = wpT[:].rearrange("(t p) m -> t p m", p=P)

        # Inputs split across BOTH HWDGE trigger queues (sync + scalar) for
        # parallel DMA rings. pair-0 qk weights lead (small), then the two
        # xT halves in parallel, then pair-0 v, then the rest.
        xT_t = xTr.rearrange("t p n -> p t n")
        wqk_t = wqkr.rearrange("t p m -> p t m")
        wv_t = wvr.rearrange("t p m -> p t m")
        nc.sync.dma_start(out=wqkT_sb[:, :, 0:P], in_=wqk_t[:, :, 0:P])
        nc.scalar.dma_start(out=wqkT_sb[:, :, P:256], in_=wqk_t[:, :, P:256])
        half = KT // 2
        nc.sync.dma_start(out=xT_sb[:, 0:half], in_=xT_t[:, 0:half])
        nc.scalar.dma_start(out=xT_sb[:, half:KT], in_=xT_t[:, half:KT])
        nc.sync.dma_start(out=wvT_sb[:, :, 0:P], in_=wv_t[:, :, 0:P])
        for t in range(1, NPAIR):
            eng = nc.sync if t % 2 else nc.scalar
            eng.dma_start(
                out=wqkT_sb[:, :, t * 256:(t + 1) * 256],
                in_=wqk_t[:, :, t * 256:(t + 1) * 256],
            )
            eng.dma_start(
                out=wvT_sb[:, :, t * P:(t + 1) * P],
                in_=wv_t[:, :, t * P:(t + 1) * P],
            )
        nc.scalar.dma_start(out=wpT_sb, in_=wpr.rearrange("t p m -> p t m"))

        # ---- PE work generators (filler units of ~0.5-1.7us of matmuls) ----
        def gen_qk(t):
            """qk pair-tile t -> qkT_sb[:, 2t] (q) and [:, 2t+1] (k)."""
            for which in range(2):
                for lo, hi in _chunks(N, 512):
                    ps = s1ps.tile([P, 512], FP, tag="s1")
                    for k in range(KT):
                        nc.tensor.matmul(
                            ps,
                            wqkT_sb[:, k, t * 256 + which * P:
                                    t * 256 + (which + 1) * P],
                            xT_sb[:, k, lo:hi],
                            start=(k == 0),
                            stop=(k == KT - 1),
                        )
                    nc.vector.tensor_copy(
                        out=qkT_sb[:, 2 * t + which, lo:hi], in_=ps)
                    yield

        def gen_v(t):
            """v pair-slice t -> v_sb[:, :, 2t:2t+2, 0:D]."""
            for half in range(2):
                ps = s1ps.tile([P, 512], FP, tag="s1")
                for jj in range(4):
                    j = half * 4 + jj
                    for k in range(KT):
                        nc.tensor.matmul(
                            ps[:, jj * P:(jj + 1) * P],
                            xT_sb[:, k, j * P:(j + 1) * P],
                            wvT_sb[:, k, t * P:(t + 1) * P],
                            start=(k == 0),
                            stop=(k == KT - 1),
                        )
                    yield
                nc.vector.tensor_copy(
                    out=v_sb[:, half * 4:(half + 1) * 4, 2 * t:2 * t + 2, 0:D],
                    in_=ps.rearrange("p (j g d) -> p j g d", g=2, d=D),
                )

        def gen_proj_partial():
            """Output-projection contributions of k-tiles 0..4 (pairs 0-4),
            SBUF-accumulated into y_acc; runs as PE filler during pair 5 so
            only the thin k=5 pass remains after the last head."""
            for i in range(NT):
                for lo, hi in _chunks(DIM, 512):
                    ps = s1ps.tile([P, 512], FP, tag="s1")
                    for k in range(KT - 1):
                        nc.tensor.matmul(
                            ps[:, 0:hi - lo],
                            oT_sb[:, k, i * P:(i + 1) * P],
                            wpT_sb[:, k, lo:hi],
                            start=(k == 0),
                            stop=(k == KT - 2),
                        )
                    nc.vector.tensor_tensor(
                        out=y_acc[:, i, lo:hi], in0=ps[:, 0:hi - lo],
                        in1=bias_sb[:, lo:hi], op=Add,
                    )
                    yield

        # ---- paired attention primitives ----
        def issue_st_pair(t, j):
            """Concurrent row-tiled S^T matmuls for heads (2t, 2t+1), key
            block j. Head 2t reads partitions 0:64, head 2t+1 reads 64:128
            -> tile_position (0,0) / (64,0) auto-derived; adjacent issue
            order makes the two K=64 matmuls execute concurrently."""
            sta = stps.tile([P, N], FP, tag="st")
            stb = stps.tile([P, N], FP, tag="st")
            for lo, hi in _chunks(N, 512):
                for hp, st in ((0, sta), (D, stb)):
                    nc.tensor.matmul(
                        st[:, lo:hi],
                        qkT_sb[hp:hp + D, 2 * t + 1, j * P:(j + 1) * P],
                        qkT_sb[hp:hp + D, 2 * t, lo:hi],
                        start=True,
                        stop=True,
                    )
            return sta, stb

        def exp_into(st, pool, tag):
            ex = pool.tile([P, N], MMDT, tag=tag)
            nc.scalar.activation(out=ex, in_=st, func=Exp, scale=float(SCALE))
            return ex

        def issue_pv(h, j, ex, ots):
            for c, (lo, hi) in enumerate(_chunks(N, 512)):
                nc.tensor.matmul(
                    ots[c],
                    v_sb[:, j, h, :],
                    ex[:, lo:hi],
                    start=(j == 0),
                    stop=(j == NT - 1),
                )

        def rchain(h, ots, pull):
            """softmax denominator row (l, at OT row D) -> reciprocal
            broadcast to all partitions via rank-1 ones x l matmuls (each
            512-chunk borrows the s1 PSUM slot in turn) -> normalize fused
            into the OT evacuation (bf16 out into oT_sb)."""
            t, hp = divmod(h, 2)
            hp *= D
            rb_sb = rp.tile([P, N], FP, tag="rb")
            for c, (lo, hi) in enumerate(_chunks(N, 512)):
                lrow = rp.tile([1, 512], mybir.dt.float32r,
                               tag=f"lrow{c}")
                nc.vector.tensor_copy(out=lrow, in_=ots[c][D:D + 1, :])
                lbc = s1ps.tile([P, 512], FP, tag="s1")
                nc.tensor.matmul(lbc, ones_f32r, lrow, start=True, stop=True)
                nc.vector.reciprocal_approx_fast(
                    out=rb_sb[:, lo:hi], in_=lbc)
                nc.vector.tensor_tensor(
                    out=oT_sb[hp:hp + D, t, lo:hi], in0=ots[c][0:D],
                    in1=rb_sb[0:D, lo:hi], op=Mult,
                )
                pull()

        # ---- interleaved pair loop ----
        def filler_for_pair(t):
            if t + 1 < NPAIR:
                def units():
                    yield from gen_qk(t + 1)
                    yield from gen_v(t + 1)
                for i, u in enumerate(units()):
                    yield u
                    if i % 4 == 3:
                        yield None  # pacing skip
            else:
                for u in gen_proj_partial():
                    yield u

        def pair_attn(t, sta, stb, filler):
            """Attention for head pair t; j-synchronized heads with
            concurrent STs; PV for head 2t inline, head 2t+1 drained at the
            boundary from SBUF-buffered exps. Returns next pair's first ST
            tiles (issued at the boundary so ScalarE never starves)."""
            a, b = 2 * t, 2 * t + 1

            def pull():
                try:
                    next(filler)
                except StopIteration:
                    pass

            exbs = []
            ota = (otps.tile([D + 1, 512], FP, tag="ot", name="ota0"),
                   otps.tile([D + 1, 512], FP, tag="ot", name="ota1"))
            for j in range(NT):
                exa = exp_into(sta, expa_p, "exa")
                exbs.append(exp_into(stb, expb_p, "exb"))
                # PE filler BEFORE the next ST pair: the in-order PE queue
                # must have ready work while the WAR on this j's exps clears
                pull()
                if j + 1 < NT:
                    sta, stb = issue_st_pair(t, j + 1)
                issue_pv(a, j, exa, ota)
                pull()
            rchain(a, ota, pull)
            # ---- pair boundary ----
            nxt = issue_st_pair(t + 1, 0) if t + 1 < NPAIR else (None, None)
            otb = (otps.tile([D + 1, 512], FP, tag="ot", name="otb0"),
                   otps.tile([D + 1, 512], FP, tag="ot", name="otb1"))
            for j in range(NT):
                issue_pv(b, j, exbs[j], otb)
                pull()
            rchain(b, otb, pull)
            return nxt

        for _ in gen_qk(0):
            pass
        sta, stb = issue_st_pair(0, 0)
        for _ in gen_v(0):
            pass
        for t in range(NPAIR):
            f = filler_for_pair(t)
            sta, stb = pair_attn(t, sta, stb, f)
            for _ in f:
                pass

      # -------- stage 3: last projection k-tile (5) + combine --------
      with (
            tc.tile_pool(name="s3y", bufs=4) as s3y,
            tc.tile_pool(name="s3ps", bufs=2, space="PSUM") as s3ps,
      ):
            yr = y[:].rearrange("(i p) e -> i p e", p=P)
            for i in range(NT):
                ps = s3ps.tile([P, DIM], FP, tag="y")
                for lo, hi in _chunks(DIM, 512):
                    nc.tensor.matmul(
                        ps[:, lo:hi],
                        oT_sb[:, KT - 1, i * P:(i + 1) * P],
                        wpT_sb[:, KT - 1, lo:hi],
                        start=True,
                        stop=True,
                    )
                y_sb = s3y.tile([P, DIM], FP, tag="ysb")
                nc.vector.tensor_tensor(
                    out=y_sb, in0=ps, in1=y_acc[:, i], op=Add,
                )
                eng = nc.sync if i % 2 == 0 else nc.scalar
                eng.dma_start(out=yr[i], in_=y_sb)


def prep_inputs(x, w_qkv, w_proj, b_proj):
    x = np.asarray(x, dtype=np.float32)
    w_qkv = np.asarray(w_qkv, dtype=np.float32)
    w_proj = np.asarray(w_proj, dtype=np.float32)
    b_proj = np.asarray(b_proj, dtype=np.float32)

    w_r = w_qkv.reshape(H, D, 3, DIM)  # rows ordered (h, d, qkv)
    wq = w_r[:, :, 0, :].reshape(DIM, DIM)  # rows (h, d)
    wk = w_r[:, :, 1, :].reshape(DIM, DIM)
    wv = w_r[:, :, 2, :].reshape(DIM, DIM)
    # pair-blocked qk: columns [q_t (128) | k_t (128)] for t = 0..5
    wqk_pairs = np.empty((2 * DIM, DIM), dtype=np.float32)
    for t in range(NPAIR):
        wqk_pairs[t * 256:t * 256 + P] = wq[t * P:(t + 1) * P]
        wqk_pairs[t * 256 + P:(t + 1) * 256] = wk[t * P:(t + 1) * P]
    wqkT = np.ascontiguousarray(wqk_pairs.T).astype(NP_MMDT)    # [768, 1536]
    wvT = np.ascontiguousarray(wv.T).astype(NP_MMDT)            # [768, 768]
    wpT = np.ascontiguousarray(w_proj.T).astype(NP_MMDT)        # [768, 768]
    xT = np.ascontiguousarray(x.transpose(0, 2, 1)).astype(NP_MMDT)
    bias = np.ascontiguousarray(b_proj.reshape(1, DIM))
    return xT, wqkT, wvT, wpT, bias


_NC = None
last_results = None


def get_nc():
    global _NC
    if _NC is None:
        _NC = build_nc()
    return _NC


def kernel(x, w_qkv, w_proj, b_proj):
    global last_results
    from concourse.bass_utils import run_bass_kernel_spmd

    nc = get_nc()
    xT, wqkT, wvT, wpT, bias = prep_inputs(x, w_qkv, w_proj, b_proj)
    in_maps = [
        {"xT": xT[c], "wqkT": wqkT, "wvT": wvT, "wpT": wpT, "bias": bias}
        for c in range(B)
    ]
    res = run_bass_kernel_spmd(nc, in_maps, core_ids=list(range(B)))
    last_results = res
    return np.stack([res.results[c]["y"] for c in range(B)], axis=0)


# revision 12
# speedup vs baseline: 1.0801x; 1.0801x over previous
"""Multi-head attention (B=8, N=1024, DIM=768, H=12) on 8 Trainium2 NeuronCores.

Sharding: pure data-parallel over the batch dimension — core c computes batch
element c end-to-end (qkv projection, softmax attention, output projection).
No collectives needed.

Numerics: matmul inputs in bf16 (x, weights, q/k, v, exp(P)) with fp32 PSUM
accumulation; softmax denominator, reciprocal, normalization and bias in
fp32.

Schedule (v3): heads are processed in PAIRS (2t, 2t+1), j-synchronized, so
the two K=64 QK^T matmuls land on disjoint row-groups of the PE array
(head 2t lives on partitions 0-63, head 2t+1 on 64-127 of the qk pair
tile). Issued back-to-back they execute CONCURRENTLY (row tiling via
auto-derived tile_position (0,0)/(64,0)), halving QK^T time - the single
largest PE consumer in the v2 trace (53.8us of 192us PE-busy).

  per pair t, per j: ONE merged ST tile [128, 2048] (head a cols 0:1024,
  head b 1024:2048) -> ONE merged exp ACTIVATE (N=2048, halves ACT
  overhead, and gives the four ST matmuls of the next j a single WAR
  release so the row-tiled (a,b) chunk pairs truly run concurrently);
  PV_a(j) into OT_a inline. The merged exp tiles are buffered in SBUF;
  PV_b for all j drains at the pair boundary (after rchain_a frees OT
  banks) so PSUM stays within 8 banks: st(4) + ot(3) + s1(1); the rchain
  lbc broadcasts borrow the s1 slot.

  stage-1 (qkv projection) for pair t+1 runs as PE filler inside pair t's
  loop; output projection partials run as filler inside pair 5; only the
  thin k=5 proj pass remains at the end.

DMA: inputs split across both HWDGE queues (sync+scalar) with xT halves in
parallel so the first matmul starts ~8us earlier; y output rows alternate
queues.
"""

import os
import sys

for _p in ("/opt/trn_rl_repo",):
    if _p not in sys.path:
        sys.path.insert(0, _p)

import ml_dtypes
import numpy as np

import concourse.bass as bass
import concourse.tile as tile
from concourse import bacc, mybir

B, N, DIM, H = 8, 1024, 768, 12
D = DIM // H  # 64
SCALE = D ** -0.5
P = 128
KT = DIM // P        # 6 contraction tiles over dim
NT = N // P          # 8 tiles over sequence
NPAIR = H // 2       # 6 head pairs
FP = mybir.dt.float32
BF = mybir.dt.bfloat16
MMDT = BF
NP_MMDT = ml_dtypes.bfloat16


def _chunks(total, size):
    return [(lo, min(lo + size, total)) for lo in range(0, total, size)]


def build_nc():
    nc = bacc.Bacc(None, target_bir_lowering=False)
    xT = nc.dram_tensor("xT", [DIM, N], MMDT, kind="ExternalInput")
    # wqkT columns are pair-blocked: [q_t | k_t] of 128 cols each, t=0..5
    wqkT = nc.dram_tensor("wqkT", [DIM, 2 * DIM], MMDT, kind="ExternalInput")
    wvT = nc.dram_tensor("wvT", [DIM, DIM], MMDT, kind="ExternalInput")
    wpT = nc.dram_tensor("wpT", [DIM, DIM], MMDT, kind="ExternalInput")
    bias = nc.dram_tensor("bias", [1, DIM], FP, kind="ExternalInput")
    y = nc.dram_tensor("y", [N, DIM], FP, kind="ExternalOutput")

    with tile.TileContext(nc) as tc:
        with nc.allow_low_precision(reason="bf16 matmul inputs"):
            _body(tc, xT, wqkT, wvT, wpT, bias, y)
    nc.compile()
    return nc


def _body(tc, xT, wqkT, wvT, wpT, bias, y):
    nc = tc.nc
    Exp = mybir.ActivationFunctionType.Exp
    Mult = mybir.AluOpType.mult
    Add = mybir.AluOpType.add

    from contextlib import ExitStack
    with tc.tile_pool(name="persist", bufs=1) as persist:
      with ExitStack() as s12:
        s1w = s12.enter_context(tc.tile_pool(name="s1w", bufs=1))
        expp = s12.enter_context(tc.tile_pool(name="expp", bufs=9))
        rp = s12.enter_context(tc.tile_pool(name="rp", bufs=2))
        s1ps = s12.enter_context(tc.tile_pool(name="s1ps", bufs=1, space="PSUM"))
        stps = s12.enter_context(tc.tile_pool(name="stps", bufs=1, space="PSUM"))
        otps = s12.enter_context(tc.tile_pool(name="otps", bufs=3, space="PSUM"))

        # qkT_sb tile index 2t = q of pair t, 2t+1 = k of pair t; rows (h%2,d)
        qkT_sb = persist.tile([P, 2 * KT, N], MMDT)     # 24 KB/part
        v_sb = persist.tile([P, NT, H, D + 1], MMDT)    # 12.7 KB/part
        oT_sb = persist.tile([P, KT, N], MMDT)          # 12 KB/part
        bias_sb = persist.tile([P, DIM], FP)            # 3 KB/part
        y_acc = persist.tile([P, NT, DIM], FP)          # 24 KB/part
        ones_f32r = persist.tile([1, P], mybir.dt.float32r)
        ones_stg = persist.tile([1, P], FP)
        nc.sync.dma_start(out=bias_sb, in_=bias[:].to_broadcast((P, DIM)))
        nc.vector.memset(v_sb[:, :, :, D], 1.0)
        nc.vector.memset(ones_stg, 1.0)
        nc.vector.tensor_copy(out=ones_f32r, in_=ones_stg)

        xT_sb = s1w.tile([P, KT, N], MMDT)              # 12 KB/part
        wqkT_sb = s1w.tile([P, KT, 2 * DIM], MMDT)      # 18 KB/part
        wvT_sb = s1w.tile([P, KT, DIM], MMDT)           # 9 KB/part
        wpT_sb = s1w.tile([P, KT, DIM], MMDT)           # 9 KB/part

        xTr = xT[:].rearrange("(t p) n -> t p n", p=P)
        wqkr = wqkT[:].rearrange("(t p) m -> t p m", p=P)
        wvr = wvT[:].rearrange("(t p) m -> t p m", p=P)
        wpr = wpT[:].rearrange("(t p) m -> t p m", p=P)

        # Inputs split across BOTH HWDGE trigger queues (sync + scalar) for
        # parallel DMA rings. pair-0 qk weights lead (small), then the two
        # xT halves in parallel, then pair-0 v, then the rest.
        xT_t = xTr.rearrange("t p n -> p t n")
        wqk_t = wqkr.rearrange("t p m -> p t m")
        wv_t = wvr.rearrange("t p m -> p t m")
        nc.sync.dma_start(out=wqkT_sb[:, :, 0:P], in_=wqk_t[:, :, 0:P])
        nc.scalar.dma_start(out=wqkT_sb[:, :, P:256], in_=wqk_t[:, :, P:256])
        half = KT // 2
        nc.sync.dma_start(out=xT_sb[:, 0:half], in_=xT_t[:, 0:half])
        nc.scalar.dma_start(out=xT_sb[:, half:KT], in_=xT_t[:, half:KT])
        nc.sync.dma_start(out=wvT_sb[:, :, 0:P], in_=wv_t[:, :, 0:P])
        for t in range(1, NPAIR):
            eng = nc.sync if t % 2 else nc.scalar
            eng.dma_start(
                out=wqkT_sb[:, :, t * 256:(t + 1) * 256],
                in_=wqk_t[:, :, t * 256:(t + 1) * 256],
            )
            eng.dma_start(
                out=wvT_sb[:, :, t * P:(t + 1) * P],
                in_=wv_t[:, :, t * P:(t + 1) * P],
            )
        nc.scalar.dma_start(out=wpT_sb, in_=wpr.rearrange("t p m -> p t m"))

        # ---- PE work generators (filler units of ~0.5-1.7us of matmuls) ----
        def gen_qk(t):
            """qk pair-tile t -> qkT_sb[:, 2t] (q) and [:, 2t+1] (k)."""
            for which in range(2):
                for lo, hi in _chunks(N, 512):
                    ps = s1ps.tile([P, 512], FP, tag="s1")
                    for k in range(KT):
                        nc.tensor.matmul(
                            ps,
                            wqkT_sb[:, k, t * 256 + which * P:
                                    t * 256 + (which + 1) * P],
                            xT_sb[:, k, lo:hi],
                            start=(k == 0),
                            stop=(k == KT - 1),
                        )
                    nc.vector.tensor_copy(
                        out=qkT_sb[:, 2 * t + which, lo:hi], in_=ps)
                    yield

        def gen_v(t):
            """v pair-slice t -> v_sb[:, :, 2t:2t+2, 0:D]."""
            for half in range(2):
                ps = s1ps.tile([P, 512], FP, tag="s1")
                for jj in range(4):
                    j = half * 4 + jj
                    for k in range(KT):
                        nc.tensor.matmul(
                            ps[:, jj * P:(jj + 1) * P],
                            xT_sb[:, k, j * P:(j + 1) * P],
                            wvT_sb[:, k, t * P:(t + 1) * P],
                            start=(k == 0),
                            stop=(k == KT - 1),
                        )
                    yield
                nc.vector.tensor_copy(
                    out=v_sb[:, half * 4:(half + 1) * 4, 2 * t:2 * t + 2, 0:D],
                    in_=ps.rearrange("p (j g d) -> p j g d", g=2, d=D),
                )

        def gen_proj_partial():
            """Output-projection contributions of k-tiles 0..4 (pairs 0-4),
            SBUF-accumulated into y_acc; runs as PE filler during pair 5 so
            only the thin k=5 pass remains after the last head."""
            for i in range(NT):
                for lo, hi in _chunks(DIM, 512):
                    ps = s1ps.tile([P, 512], FP, tag="s1")
                    for k in range(KT - 1):
                        nc.tensor.matmul(
                            ps[:, 0:hi - lo],
                            oT_sb[:, k, i * P:(i + 1) * P],
                            wpT_sb[:, k, lo:hi],
                            start=(k == 0),
                            stop=(k == KT - 2),
                        )
                    nc.vector.tensor_tensor(
                        out=y_acc[:, i, lo:hi], in0=ps[:, 0:hi - lo],
                        in1=bias_sb[:, lo:hi], op=Add,
                    )
                    yield

        # ---- paired attention primitives ----
        def issue_st_pair(t, j):
            """Concurrent row-tiled S^T matmuls for heads (2t, 2t+1), key
            block j, into ONE merged [128, 2048] PSUM tile (head a cols
            0:1024, head b cols 1024:2048). The single tile means a single
            WAR release (the merged exp), so all four matmuls become ready
            together and the (a,b) chunk pairs - which live on disjoint
            row-groups via auto tile_position (0,0)/(64,0) - genuinely
            execute concurrently."""
            st = stps.tile([P, 2 * N], FP, tag="st")
            for lo, hi in _chunks(N, 512):
                for hp, off in ((0, 0), (D, N)):
                    nc.tensor.matmul(
                        st[:, off + lo:off + hi],
                        qkT_sb[hp:hp + D, 2 * t + 1, j * P:(j + 1) * P],
                        qkT_sb[hp:hp + D, 2 * t, lo:hi],
                        start=True,
                        stop=True,
                    )
            return st

        def exp_pair(st):
            """One ACTIVATE over the merged ST pair tile: exp for both
            heads in a single N=2048 instruction (halves ACT overhead)."""
            ex = expp.tile([P, 2 * N], MMDT, tag="exab")
            nc.scalar.activation(out=ex, in_=st, func=Exp, scale=float(SCALE))
            return ex

        def issue_pv(h, j, ex, ots, off):
            for c, (lo, hi) in enumerate(_chunks(N, 512)):
                nc.tensor.matmul(
                    ots[c],
                    v_sb[:, j, h, :],
                    ex[:, off + lo:off + hi],
                    start=(j == 0),
                    stop=(j == NT - 1),
                )

        def rchain(h, ots, pull):
            """softmax denominator row (l, at OT row D) -> reciprocal
            broadcast to all partitions via rank-1 ones x l matmuls (each
            512-chunk borrows the s1 PSUM slot in turn) -> normalize fused
            into the OT evacuation (bf16 out into oT_sb)."""
            t, hp = divmod(h, 2)
            hp *= D
            rb_sb = rp.tile([P, N], FP, tag="rb")
            for c, (lo, hi) in enumerate(_chunks(N, 512)):
                lrow = rp.tile([1, 512], mybir.dt.float32r,
                               tag=f"lrow{c}")
                nc.vector.tensor_copy(out=lrow, in_=ots[c][D:D + 1, :])
                lbc = s1ps.tile([P, 512], FP, tag="s1")
                nc.tensor.matmul(lbc, ones_f32r, lrow, start=True, stop=True)
                nc.vector.reciprocal_approx_fast(
                    out=rb_sb[:, lo:hi], in_=lbc)
                nc.vector.tensor_tensor(
                    out=oT_sb[hp:hp + D, t, lo:hi], in0=ots[c][0:D],
                    in1=rb_sb[0:D, lo:hi], op=Mult,
                )
                pull()

        # ---- interleaved pair loop ----
        def filler_for_pair(t):
            if t + 1 < NPAIR:
                def units():
                    yield from gen_qk(t + 1)
                    yield from gen_v(t + 1)
                for i, u in enumerate(units()):
                    yield u
                    if i % 4 == 3:
                        yield None  # pacing skip
            else:
                for u in gen_proj_partial():
                    yield u

        def pair_attn(t, st, filler):
            """Attention for head pair t; j-synchronized heads with
            concurrent STs; PV for head 2t inline, head 2t+1 drained at the
            boundary from the SBUF-buffered merged exp tiles. Returns the
            next pair's first merged ST tile (issued at the boundary so
            ScalarE never starves)."""
            a, b = 2 * t, 2 * t + 1

            def pull():
                try:
                    next(filler)
                except StopIteration:
                    pass

            exs = []
            ota = (otps.tile([D + 1, 512], FP, tag="ot", name="ota0"),
                   otps.tile([D + 1, 512], FP, tag="ot", name="ota1"))
            for j in range(NT):
                exs.append(exp_pair(st))
                # PE filler BEFORE the next ST pair: the in-order PE queue
                # must have ready work while the WAR on this j's exp clears
                pull()
                if j + 1 < NT:
                    st = issue_st_pair(t, j + 1)
                issue_pv(a, j, exs[j], ota, 0)
                pull()
            rchain(a, ota, pull)
            # ---- pair boundary ----
            nxt = issue_st_pair(t + 1, 0) if t + 1 < NPAIR else None
            otb = (otps.tile([D + 1, 512], FP, tag="ot", name="otb0"),
                   otps.tile([D + 1, 512], FP, tag="ot", name="otb1"))
            for j in range(NT):
                issue_pv(b, j, exs[j], otb, N)
                pull()
            rchain(b, otb, pull)
            return nxt

        for _ in gen_qk(0):
            pass
        st = issue_st_pair(0, 0)
        for _ in gen_v(0):
            pass
        for t in range(NPAIR):
            f = filler_for_pair(t)
            st = pair_attn(t, st, f)
            for _ in f:
                pass

      # -------- stage 3: last projection k-tile (5) + combine --------
      with (
            tc.tile_pool(name="s3y", bufs=4) as s3y,
            tc.tile_pool(name="s3ps", bufs=2, space="PSUM") as s3ps,
      ):
            yr = y[:].rearrange("(i p) e -> i p e", p=P)
            for i in range(NT):
                ps = s3ps.tile([P, DIM], FP, tag="y")
                for lo, hi in _chunks(DIM, 512):
                    nc.tensor.matmul(
                        ps[:, lo:hi],
                        oT_sb[:, KT - 1, i * P:(i + 1) * P],
                        wpT_sb[:, KT - 1, lo:hi],
                        start=True,
                        stop=True,
                    )
                y_sb = s3y.tile([P, DIM], FP, tag="ysb")
                nc.vector.tensor_tensor(
                    out=y_sb, in0=ps, in1=y_acc[:, i], op=Add,
                )
                eng = nc.sync if i % 2 == 0 else nc.scalar
                eng.dma_start(out=yr[i], in_=y_sb)


def prep_inputs(x, w_qkv, w_proj, b_proj):
    x = np.asarray(x, dtype=np.float32)
    w_qkv = np.asarray(w_qkv, dtype=np.float32)
    w_proj = np.asarray(w_proj, dtype=np.float32)
    b_proj = np.asarray(b_proj, dtype=np.float32)

    w_r = w_qkv.reshape(H, D, 3, DIM)  # rows ordered (h, d, qkv)
    wq = w_r[:, :, 0, :].reshape(DIM, DIM)  # rows (h, d)
    wk = w_r[:, :, 1, :].reshape(DIM, DIM)
    wv = w_r[:, :, 2, :].reshape(DIM, DIM)
    # pair-blocked qk: columns [q_t (128) | k_t (128)] for t = 0..5
    wqk_pairs = np.empty((2 * DIM, DIM), dtype=np.float32)
    for t in range(NPAIR):
        wqk_pairs[t * 256:t * 256 + P] = wq[t * P:(t + 1) * P]
        wqk_pairs[t * 256 + P:(t + 1) * 256] = wk[t * P:(t + 1) * P]
    wqkT = np.ascontiguousarray(wqk_pairs.T).astype(NP_MMDT)    # [768, 1536]
    wvT = np.ascontiguousarray(wv.T).astype(NP_MMDT)            # [768, 768]
    wpT = np.ascontiguousarray(w_proj.T).astype(NP_MMDT)        # [768, 768]
    xT = np.ascontiguousarray(x.transpose(0, 2, 1)).astype(NP_MMDT)
    bias = np.ascontiguousarray(b_proj.reshape(1, DIM))
    return xT, wqkT, wvT, wpT, bias


_NC = None
last_results = None


def get_nc():
    global _NC
    if _NC is None:
        _NC = build_nc()
    return _NC


def kernel(x, w_qkv, w_proj, b_proj):
    global last_results
    from concourse.bass_utils import run_bass_kernel_spmd

    nc = get_nc()
    xT, wqkT, wvT, wpT, bias = prep_inputs(x, w_qkv, w_proj, b_proj)
    in_maps = [
        {"xT": xT[c], "wqkT": wqkT, "wvT": wvT, "wpT": wpT, "bias": bias}
        for c in range(B)
    ]
    res = run_bass_kernel_spmd(nc, in_maps, core_ids=list(range(B)))
    last_results = res
    return np.stack([res.results[c]["y"] for c in range(B)], axis=0)


# revision 14
# speedup vs baseline: 1.1586x; 1.0726x over previous
"""Multi-head attention (B=8, N=1024, DIM=768, H=12) on 8 Trainium2 NeuronCores.

Sharding: pure data-parallel over the batch dimension — core c computes batch
element c end-to-end (qkv projection, softmax attention, output projection).
No collectives needed.

Numerics: matmul inputs in bf16 (x, weights, q/k, v, exp(P), softmax
denominator row for the rank-1 broadcast) with fp32 PSUM accumulation;
reciprocal, normalization and bias in fp32.

Schedule (v5): heads are processed in PAIRS (2t, 2t+1), j-synchronized.
The two K=64 QK^T matmuls of a pair land on disjoint row-groups of the PE
array (head 2t on partitions 0-63, head 2t+1 on 64-127 of the qk pair
tile) and execute CONCURRENTLY (row tiling via auto tile_position
(0,0)/(64,0)) when issued back-to-back with a common dependency release.

  Per (pair, j) the S^T block is built as TWO staggered [128, 1024] PSUM
  tiles, each holding one 512-wide i-chunk of BOTH heads (head a cols
  0:512, head b 512:1024). Each tile gets its own exp ACTIVATE, so the
  WAR release for the next j's ST chunk-pair fires as soon as ITS exp
  completes - the c0/c1 stagger means the PE never waits a full exp and
  ScalarE never gaps. PV for head 2t runs inline into OT_a; the exp
  tiles are buffered in SBUF and PV for head 2t+1 drains at the pair
  boundary (after rchain_a frees OT banks). PSUM: st(2x2) + ot(3) +
  s1(1) = 8 banks; the rchain lbc broadcasts borrow the s1 slot.

  Stage-1 (qkv projection) for pair t+1 runs as PE filler inside pair t;
  output-projection k-chains run as filler once their oT k-tiles exist
  (k0-2 during pair 3, k3-4 during pair 5, k5 after the last rchain).

DMA: all inputs are pre-arranged host-side into the exact [128, X] SBUF
layout so every transfer is one contiguous span per partition (large
descriptors); split across both HWDGE queues (sync + scalar) with the two
xT halves in parallel. y output rows alternate queues.
"""

import os
import sys

for _p in ("/opt/trn_rl_repo",):
    if _p not in sys.path:
        sys.path.insert(0, _p)

import ml_dtypes
import numpy as np

import concourse.bass as bass
import concourse.tile as tile
from concourse import bacc, mybir

B, N, DIM, H = 8, 1024, 768, 12
D = DIM // H  # 64
SCALE = D ** -0.5
P = 128
KT = DIM // P        # 6 contraction tiles over dim
NT = N // P          # 8 tiles over sequence
NPAIR = H // 2       # 6 head pairs
FP = mybir.dt.float32
BF = mybir.dt.bfloat16
MMDT = BF
NP_MMDT = ml_dtypes.bfloat16
HC = 512             # i-chunk width (PSUM bank)


def _chunks(total, size):
    return [(lo, min(lo + size, total)) for lo in range(0, total, size)]


def build_nc():
    nc = bacc.Bacc(None, target_bir_lowering=False)
    # flat [128, X] layouts, one contiguous span per partition per transfer
    xT = nc.dram_tensor("xT", [P, KT * N], MMDT, kind="ExternalInput")
    wqk = nc.dram_tensor("wqk", [P, NPAIR * KT * 256], MMDT,
                         kind="ExternalInput")
    wv = nc.dram_tensor("wv", [P, NPAIR * KT * P], MMDT, kind="ExternalInput")
    wp = nc.dram_tensor("wp", [P, KT * DIM], MMDT, kind="ExternalInput")
    bias = nc.dram_tensor("bias", [1, DIM], FP, kind="ExternalInput")
    y = nc.dram_tensor("y", [N, DIM], FP, kind="ExternalOutput")

    with tile.TileContext(nc) as tc:
        with nc.allow_low_precision(reason="bf16 matmul inputs"):
            _body(tc, xT, wqk, wv, wp, bias, y)
    nc.compile()
    return nc


def _body(tc, xT, wqk, wv, wp, bias, y):
    nc = tc.nc
    Exp = mybir.ActivationFunctionType.Exp
    Mult = mybir.AluOpType.mult
    Add = mybir.AluOpType.add

    from contextlib import ExitStack
    with tc.tile_pool(name="persist", bufs=1) as persist:
      with ExitStack() as s12:
        s1w = s12.enter_context(tc.tile_pool(name="s1w", bufs=1))
        expp = s12.enter_context(tc.tile_pool(name="expp", bufs=9))
        rp = s12.enter_context(tc.tile_pool(name="rp", bufs=2))
        s1ps = s12.enter_context(tc.tile_pool(name="s1ps", bufs=1, space="PSUM"))
        stps = s12.enter_context(tc.tile_pool(name="stps", bufs=2, space="PSUM"))
        otps = s12.enter_context(tc.tile_pool(name="otps", bufs=3, space="PSUM"))

        # qkT_sb tile index 2t = q of pair t, 2t+1 = k of pair t; rows (h%2,d)
        qkT_sb = persist.tile([P, 2 * KT, N], MMDT)     # 24 KB/part
        v_sb = persist.tile([P, NT, H, D + 1], MMDT)    # 12.7 KB/part
        oT_sb = persist.tile([P, KT, N], MMDT)          # 12 KB/part
        bias_sb = persist.tile([P, DIM], FP)            # 3 KB/part
        y_acc = persist.tile([P, NT, DIM], FP)          # 24 KB/part
        ones_bf = persist.tile([1, P], MMDT)
        nc.vector.memset(v_sb[:, :, :, D], 1.0)
        nc.vector.memset(ones_bf, 1.0)

        xT_sb = s1w.tile([P, KT, N], MMDT)              # 12 KB/part
        wqkT_sb = s1w.tile([P, NPAIR, KT, 256], MMDT)   # 18 KB/part
        wvT_sb = s1w.tile([P, NPAIR, KT, P], MMDT)      # 9 KB/part
        wpT_sb = s1w.tile([P, KT, DIM], MMDT)           # 9 KB/part

        # ---- input DMAs: both queues in parallel, large flat descriptors
        wqk_r = wqk[:].rearrange("p (t k m) -> p t k m", t=NPAIR, k=KT)
        wv_r = wv[:].rearrange("p (t k m) -> p t k m", t=NPAIR, k=KT)
        xT_r = xT[:].rearrange("p (k n) -> p k n", k=KT)
        hk = KT // 2
        nc.sync.dma_start(out=wqkT_sb[:, 0], in_=wqk_r[:, 0])
        nc.sync.dma_start(out=xT_sb[:, 0:hk], in_=xT_r[:, 0:hk])
        nc.scalar.dma_start(out=xT_sb[:, hk:KT], in_=xT_r[:, hk:KT])
        nc.sync.dma_start(out=wvT_sb[:, 0], in_=wv_r[:, 0])
        for t in range(1, NPAIR):
            eng = nc.sync if t % 2 else nc.scalar
            eng.dma_start(out=wqkT_sb[:, t], in_=wqk_r[:, t])
            eng.dma_start(out=wvT_sb[:, t], in_=wv_r[:, t])
        nc.scalar.dma_start(
            out=wpT_sb, in_=wp[:].rearrange("p (k m) -> p k m", k=KT))
        nc.scalar.dma_start(out=bias_sb, in_=bias[:].to_broadcast((P, DIM)))

        # ---- PE work generators (filler units of ~0.5-1.7us of matmuls) ----
        def gen_qk(t):
            """qk pair-tile t -> qkT_sb[:, 2t] (q) and [:, 2t+1] (k)."""
            for which in range(2):
                for lo, hi in _chunks(N, HC):
                    ps = s1ps.tile([P, HC], FP, tag="s1")
                    for k in range(KT):
                        nc.tensor.matmul(
                            ps,
                            wqkT_sb[:, t, k, which * P:(which + 1) * P],
                            xT_sb[:, k, lo:hi],
                            start=(k == 0),
                            stop=(k == KT - 1),
                        )
                    nc.vector.tensor_copy(
                        out=qkT_sb[:, 2 * t + which, lo:hi], in_=ps)
                    yield

        def gen_v(t):
            """v pair-slice t -> v_sb[:, :, 2t:2t+2, 0:D]."""
            for half in range(2):
                ps = s1ps.tile([P, HC], FP, tag="s1")
                for jj in range(4):
                    j = half * 4 + jj
                    for k in range(KT):
                        nc.tensor.matmul(
                            ps[:, jj * P:(jj + 1) * P],
                            xT_sb[:, k, j * P:(j + 1) * P],
                            wvT_sb[:, t, k, :],
                            start=(k == 0),
                            stop=(k == KT - 1),
                        )
                    yield
                nc.vector.tensor_copy(
                    out=v_sb[:, half * 4:(half + 1) * 4, 2 * t:2 * t + 2, 0:D],
                    in_=ps.rearrange("p (j g d) -> p j g d", g=2, d=D),
                )

        def gen_proj(ks, first):
            """Output-projection contribution of oT k-tiles `ks`,
            SBUF-accumulated into y_acc (adds bias on the first round)."""
            for i in range(NT):
                for lo, hi in _chunks(DIM, HC):
                    ps = s1ps.tile([P, HC], FP, tag="s1")
                    for ki, k in enumerate(ks):
                        nc.tensor.matmul(
                            ps[:, 0:hi - lo],
                            oT_sb[:, k, i * P:(i + 1) * P],
                            wpT_sb[:, k, lo:hi],
                            start=(ki == 0),
                            stop=(ki == len(ks) - 1),
                        )
                    nc.vector.tensor_tensor(
                        out=y_acc[:, i, lo:hi], in0=ps[:, 0:hi - lo],
                        in1=bias_sb[:, lo:hi] if first else y_acc[:, i, lo:hi],
                        op=Add,
                    )
                    yield

        # ---- paired attention primitives ----
        def issue_st_half(t, j, c):
            """S^T i-chunk c for BOTH heads of pair t, key block j, into one
            [128, 1024] PSUM tile: head 2t -> cols 0:512 (partitions 0:64 of
            the qk tile), head 2t+1 -> cols 512:1024 (partitions 64:128).
            One tile = one WAR release (its exp), so the two K=64 matmuls
            stay back-to-back and run concurrently on disjoint row groups."""
            st = stps.tile([P, N], FP, tag="st")
            lo = c * HC
            for hp, off in ((0, 0), (D, HC)):
                nc.tensor.matmul(
                    st[:, off:off + HC],
                    qkT_sb[hp:hp + D, 2 * t + 1, j * P:(j + 1) * P],
                    qkT_sb[hp:hp + D, 2 * t, lo:lo + HC],
                    start=True,
                    stop=True,
                )
            return st

        def exp_half(st, c):
            ex = expp.tile([P, N], MMDT, tag=f"exc{c}")
            nc.scalar.activation(out=ex, in_=st, func=Exp, scale=float(SCALE))
            return ex

        def pv_chunk(h, j, ex, ot):
            """One PV accumulation matmul: ex cols select the head (2t ->
            0:512, 2t+1 -> 512:1024)."""
            off = (h % 2) * HC
            nc.tensor.matmul(
                ot,
                v_sb[:, j, h, :],
                ex[:, off:off + HC],
                start=(j == 0),
                stop=(j == NT - 1),
            )

        def rchain(h, ots, pull):
            """softmax denominator row (l, at OT row D) -> rank-1 ones x l
            broadcast (bf16 inputs, FWL weight load; lbc borrows the s1
            slot) -> fp32 reciprocal -> normalize fused into the OT
            evacuation (bf16 out into oT_sb)."""
            t, hp = divmod(h, 2)
            hp *= D
            rb_sb = rp.tile([P, N], FP, tag="rb")
            for c, (lo, hi) in enumerate(_chunks(N, HC)):
                lrow = rp.tile([1, HC], MMDT, tag=f"lrow{c}")
                nc.vector.tensor_copy(out=lrow, in_=ots[c][D:D + 1, :])
                lbc = s1ps.tile([P, HC], FP, tag="s1")
                nc.tensor.matmul(lbc, ones_bf, lrow, start=True, stop=True)
                nc.vector.reciprocal_approx_fast(out=rb_sb[:, lo:hi], in_=lbc)
                nc.vector.tensor_tensor(
                    out=oT_sb[hp:hp + D, t, lo:hi], in0=ots[c][0:D],
                    in1=rb_sb[0:D, lo:hi], op=Mult,
                )
                pull()

        # ---- interleaved pair loop ----
        def filler_for_pair(t):
            def units():
                if t + 1 < NPAIR:
                    yield from gen_qk(t + 1)
                    yield from gen_v(t + 1)
                if t == 3:
                    yield from gen_proj([0, 1, 2], first=True)
                elif t == 5:
                    yield from gen_proj([3, 4], first=False)
            # pacing: spread the units across the ~36 pulls of a pair so
            # late-pair pulls still find PE work
            skips = {3: 0.25, 5: 1.0}.get(t, 2.0)
            acc = 0.0
            for u in units():
                yield u
                acc += skips
                while acc >= 1.0:
                    yield None
                    acc -= 1.0

        def pair_attn(t, st2, filler):
            """Attention for head pair t; PV for head 2t inline, head 2t+1
            drained at the boundary from the SBUF-buffered exp tiles.
            Returns the next pair's first ST chunk tiles."""
            a, b = 2 * t, 2 * t + 1

            def pull():
                try:
                    next(filler)
                except StopIteration:
                    pass

            exs = []
            ota = (otps.tile([D + 1, HC], FP, tag="ot", name="ota0"),
                   otps.tile([D + 1, HC], FP, tag="ot", name="ota1"))
            for j in range(NT):
                st_c0, st_c1 = st2
                ex_c0 = exp_half(st_c0, 0)
                ex_c1 = exp_half(st_c1, 1)
                exs.append((ex_c0, ex_c1))
                pull()
                n0 = issue_st_half(t, j + 1, 0) if j + 1 < NT else None
                pv_chunk(a, j, ex_c0, ota[0])
                pull()
                n1 = issue_st_half(t, j + 1, 1) if j + 1 < NT else None
                pv_chunk(a, j, ex_c1, ota[1])
                pull()
                st2 = (n0, n1)
            rchain(a, ota, pull)
            # ---- pair boundary ----
            if t + 1 < NPAIR:
                nxt = (issue_st_half(t + 1, 0, 0), issue_st_half(t + 1, 0, 1))
            else:
                nxt = None
            otb = (otps.tile([D + 1, HC], FP, tag="ot", name="otb0"),
                   otps.tile([D + 1, HC], FP, tag="ot", name="otb1"))
            for j in range(NT):
                pv_chunk(b, j, exs[j][0], otb[0])
                pv_chunk(b, j, exs[j][1], otb[1])
                pull()
            rchain(b, otb, pull)
            return nxt

        for _ in gen_qk(0):
            pass
        st2 = (issue_st_half(0, 0, 0), issue_st_half(0, 0, 1))
        for _ in gen_v(0):
            pass
        for t in range(NPAIR):
            f = filler_for_pair(t)
            st2 = pair_attn(t, st2, f)
            for _ in f:
                pass

      # -------- stage 3: last projection k-tile (5) + combine --------
      with (
            tc.tile_pool(name="s3y", bufs=4) as s3y,
            tc.tile_pool(name="s3ps", bufs=2, space="PSUM") as s3ps,
      ):
            yr = y[:].rearrange("(i p) e -> i p e", p=P)
            for i in range(NT):
                ps = s3ps.tile([P, DIM], FP, tag="y")
                for lo, hi in _chunks(DIM, HC):
                    nc.tensor.matmul(
                        ps[:, lo:hi],
                        oT_sb[:, KT - 1, i * P:(i + 1) * P],
                        wpT_sb[:, KT - 1, lo:hi],
                        start=True,
                        stop=True,
                    )
                y_sb = s3y.tile([P, DIM], FP, tag="ysb")
                nc.vector.tensor_tensor(
                    out=y_sb, in0=ps, in1=y_acc[:, i], op=Add,
                )
                eng = nc.sync if i % 2 == 0 else nc.scalar
                eng.dma_start(out=yr[i], in_=y_sb)


def prep_inputs(x, w_qkv, w_proj, b_proj):
    x = np.asarray(x, dtype=np.float32)
    w_qkv = np.asarray(w_qkv, dtype=np.float32)
    w_proj = np.asarray(w_proj, dtype=np.float32)
    b_proj = np.asarray(b_proj, dtype=np.float32)

    w_r = w_qkv.reshape(H, D, 3, DIM)  # rows ordered (h, d, qkv)
    wq = w_r[:, :, 0, :].reshape(DIM, DIM)  # rows (h, d)
    wk = w_r[:, :, 1, :].reshape(DIM, DIM)
    wv = w_r[:, :, 2, :].reshape(DIM, DIM)
    # pair-blocked qk columns: [q_t (128) | k_t (128)] for t = 0..5
    wqk_pairs = np.empty((2 * DIM, DIM), dtype=np.float32)
    for t in range(NPAIR):
        wqk_pairs[t * 256:t * 256 + P] = wq[t * P:(t + 1) * P]
        wqk_pairs[t * 256 + P:(t + 1) * 256] = wk[t * P:(t + 1) * P]

    def flat(wT, m):  # [DIM, M] (rows = contraction) -> [P, NPAIR|1.., KT, m]
        return np.ascontiguousarray(
            wT.reshape(KT, P, wT.shape[1] // m, m).transpose(1, 2, 0, 3)
        )

    wqk_f = flat(wqk_pairs.T, 256).reshape(P, -1).astype(NP_MMDT)
    wv_f = flat(wv.T, P).reshape(P, -1).astype(NP_MMDT)
    wp_f = np.ascontiguousarray(
        w_proj.T.reshape(KT, P, DIM).transpose(1, 0, 2)
    ).reshape(P, -1).astype(NP_MMDT)
    # x: [B, N, DIM] -> per-core xT [P, KT*N] with k-tile-major layout
    xTb = x.transpose(0, 2, 1).reshape(B, KT, P, N).transpose(0, 2, 1, 3)
    xT_f = np.ascontiguousarray(xTb).reshape(B, P, KT * N).astype(NP_MMDT)
    bias = np.ascontiguousarray(b_proj.reshape(1, DIM))
    return xT_f, wqk_f, wv_f, wp_f, bias


_NC = None
last_results = None


def get_nc():
    global _NC
    if _NC is None:
        _NC = build_nc()
    return _NC


def kernel(x, w_qkv, w_proj, b_proj):
    global last_results
    from concourse.bass_utils import run_bass_kernel_spmd

    nc = get_nc()
    xT_f, wqk_f, wv_f, wp_f, bias = prep_inputs(x, w_qkv, w_proj, b_proj)
    in_maps = [
        {"xT": xT_f[c], "wqk": wqk_f, "wv": wv_f, "wp": wp_f, "bias": bias}
        for c in range(B)
    ]
    res = run_bass_kernel_spmd(nc, in_maps, core_ids=list(range(B)))
    last_results = res
    return np.stack([res.results[c]["y"] for c in range(B)], axis=0)


# revision 30
# speedup vs baseline: 1.2731x; 1.0988x over previous
"""Multi-head attention (B=8, N=1024, DIM=768, H=12) on 8 Trainium2 NeuronCores.

Sharding: pure data-parallel over the batch dimension — core c computes batch
element c end-to-end (qkv projection, softmax attention, output projection).
No collectives needed.

Numerics: matmul inputs in bf16 (x, weights, q/k, v, exp(P), softmax
denominator row for the rank-1 broadcast) with fp32 PSUM accumulation;
reciprocal, normalization and bias in fp32.

Schedule (v7): heads are processed in PAIRS (2t, 2t+1), j-synchronized.
The two K=64 QK^T matmuls of a pair land on disjoint row-groups of the PE
array (head 2t on partitions 0-63, head 2t+1 on 64-127 of the qk pair
tile) and execute CONCURRENTLY (row tiling via auto tile_position
(0,0)/(64,0)) when issued back-to-back with a common dependency release.

  Per (pair, j) the S^T block is built as TWO staggered [128, 1024] PSUM
  tiles, each holding one 512-wide i-chunk of BOTH heads (head a cols
  0:512, head b 512:1024). Each tile gets its own exp ACTIVATE, so the
  WAR release for the next j's ST chunk-pair fires as soon as ITS exp
  completes - the c0/c1 stagger means the PE never waits a full exp and
  ScalarE never gaps. PV for head 2t runs inline into OT_a; the exp
  tiles are buffered in SBUF and PV for head 2t+1 drains at the pair
  boundary (after rchain_a frees OT banks). PSUM: st(2x2) + ot(3) +
  s1(1) = 8 banks; the rchain lbc broadcasts borrow the s1 slot.

  Stage-1 (qkv projection) for pair t+1 runs as PE filler inside pair t;
  output-projection k-chains run as filler once their oT k-tiles exist
  (k0-2 during pair 3, k3-4 during pair 5, k5 after the last rchain).

DMA: all inputs are pre-arranged host-side into the exact [128, X] SBUF
layout so every transfer is one contiguous span per partition (large
descriptors); split across both HWDGE queues (sync + scalar) with the two
xT halves in parallel. y output rows alternate queues.
"""

import os
import sys

for _p in ("/opt/trn_rl_repo",):
    if _p not in sys.path:
        sys.path.insert(0, _p)

import ml_dtypes
import numpy as np

import concourse.bass as bass
import concourse.tile as tile
from concourse import bacc, mybir

B, N, DIM, H = 8, 1024, 768, 12
D = DIM // H  # 64
SCALE = D ** -0.5
P = 128
KT = DIM // P        # 6 contraction tiles over dim
NT = N // P          # 8 tiles over sequence
NPAIR = H // 2       # 6 head pairs
FP = mybir.dt.float32
BF = mybir.dt.bfloat16
MMDT = BF
NP_MMDT = ml_dtypes.bfloat16
HC = 512             # i-chunk width (PSUM bank)


def _chunks(total, size):
    return [(lo, min(lo + size, total)) for lo in range(0, total, size)]


def build_nc():
    nc = bacc.Bacc(None, target_bir_lowering=False)
    # flat [128, X] layouts, one contiguous span per partition per transfer
    xT = nc.dram_tensor("xT", [P, KT * N], MMDT, kind="ExternalInput")
    wqk = nc.dram_tensor("wqk", [P, NPAIR * KT * 256], MMDT,
                         kind="ExternalInput")
    wv = nc.dram_tensor("wv", [P, NPAIR * KT * P], MMDT, kind="ExternalInput")
    wp = nc.dram_tensor("wp", [P, KT * DIM], MMDT, kind="ExternalInput")
    bias = nc.dram_tensor("bias", [1, DIM], FP, kind="ExternalInput")
    y = nc.dram_tensor("y", [N, DIM], FP, kind="ExternalOutput")

    with tile.TileContext(nc) as tc:
        with nc.allow_low_precision(reason="bf16 matmul inputs"):
            _body(tc, xT, wqk, wv, wp, bias, y)
    nc.compile()
    return nc


def _body(tc, xT, wqk, wv, wp, bias, y):
    nc = tc.nc
    Exp = mybir.ActivationFunctionType.Exp
    Mult = mybir.AluOpType.mult
    Add = mybir.AluOpType.add

    from contextlib import ExitStack
    with tc.tile_pool(name="persist", bufs=1) as persist:
      with ExitStack() as s12:
        s1w = s12.enter_context(tc.tile_pool(name="s1w", bufs=1))
        expp = s12.enter_context(tc.tile_pool(name="expp", bufs=9))
        rp = s12.enter_context(tc.tile_pool(name="rp", bufs=2))
        ounp = s12.enter_context(tc.tile_pool(name="ounp", bufs=2))
        s1ps = s12.enter_context(tc.tile_pool(name="s1ps", bufs=1, space="PSUM"))
        stps = s12.enter_context(tc.tile_pool(name="stps", bufs=2, space="PSUM"))
        otps = s12.enter_context(tc.tile_pool(name="otps", bufs=2, space="PSUM"))

        # qkT_sb tile index 2t = q of pair t, 2t+1 = k of pair t; rows (h%2,d)
        qkT_sb = persist.tile([P, 2 * KT, N], MMDT)     # 24 KB/part
        v_sb = persist.tile([P, NT, H, D + 1], MMDT)    # 12.7 KB/part
        oT_sb = persist.tile([P, KT, N], MMDT)          # 12 KB/part
        bias_sb = persist.tile([P, DIM], FP)            # 3 KB/part
        y_acc = persist.tile([P, NT, DIM], FP)          # 24 KB/part
        ones_col = persist.tile([P, 1], MMDT)
        ones128 = persist.tile([P, P], MMDT)
        nc.vector.memset(ones_col, 1.0)
        nc.vector.memset(ones128, 1.0)

        xT_sb = s1w.tile([P, KT, N], MMDT)              # 12 KB/part
        wqkT_sb = s1w.tile([P, NPAIR, KT, 256], MMDT)   # 18 KB/part
        wvT_sb = s1w.tile([P, NPAIR, KT, P], MMDT)      # 9 KB/part
        wpT_sb = s1w.tile([P, KT, DIM], MMDT)           # 9 KB/part

        # ---- input DMAs: both queues in parallel, large flat descriptors
        wqk_r = wqk[:].rearrange("p (t k m) -> p t k m", t=NPAIR, k=KT)
        wv_r = wv[:].rearrange("p (t k m) -> p t k m", t=NPAIR, k=KT)
        xT_r = xT[:].rearrange("p (k n) -> p k n", k=KT)
        nc.sync.dma_start(out=wqkT_sb[:, 0], in_=wqk_r[:, 0])
        # xT split per k-tile across both queues: more DMA transfers in
        # flight -> earlier first-chain start
        for k in range(KT):
            eng = nc.sync if k % 2 == 0 else nc.scalar
            eng.dma_start(out=xT_sb[:, k], in_=xT_r[:, k])
        nc.sync.dma_start(out=wvT_sb[:, 0], in_=wv_r[:, 0])
        for t in range(1, NPAIR):
            eng = nc.sync if t % 2 else nc.scalar
            eng.dma_start(out=wqkT_sb[:, t], in_=wqk_r[:, t])
            eng.dma_start(out=wvT_sb[:, t], in_=wv_r[:, t])
        nc.scalar.dma_start(
            out=wpT_sb, in_=wp[:].rearrange("p (k m) -> p k m", k=KT))
        nc.scalar.dma_start(out=bias_sb, in_=bias[:].to_broadcast((P, DIM)))

        # ---- PE work generators (filler units of ~0.5-1.7us of matmuls) ----
        def gen_qk(t):
            """qk pair-tile t -> qkT_sb[:, 2t] (q) and [:, 2t+1] (k).
            Chunk-major unit order (q-c0, k-c0, q-c1, k-c1) so the first
            ST of a pair only needs the first two units."""
            for lo, hi in _chunks(N, HC):
                for which in range(2):
                    ps = s1ps.tile([P, HC], FP, tag="s1")
                    for k in range(KT):
                        nc.tensor.matmul(
                            ps,
                            wqkT_sb[:, t, k, which * P:(which + 1) * P],
                            xT_sb[:, k, lo:hi],
                            start=(k == 0),
                            stop=(k == KT - 1),
                        )
                    nc.vector.tensor_copy(
                        out=qkT_sb[:, 2 * t + which, lo:hi], in_=ps)
                    yield

        def gen_v(t):
            """v pair-slice t -> v_sb[:, :, 2t:2t+2, 0:D]."""
            for half in range(2):
                ps = s1ps.tile([P, HC], FP, tag="s1")
                for jj in range(4):
                    j = half * 4 + jj
                    for k in range(KT):
                        nc.tensor.matmul(
                            ps[:, jj * P:(jj + 1) * P],
                            xT_sb[:, k, j * P:(j + 1) * P],
                            wvT_sb[:, t, k, :],
                            start=(k == 0),
                            stop=(k == KT - 1),
                        )
                    yield
                nc.vector.tensor_copy(
                    out=v_sb[:, half * 4:(half + 1) * 4, 2 * t:2 * t + 2, 0:D],
                    in_=ps.rearrange("p (j g d) -> p j g d", g=2, d=D),
                )

        def gen_proj(ks, first):
            """Output-projection contribution of oT k-tiles `ks`,
            SBUF-accumulated into y_acc (adds bias on the first round)."""
            for i in range(NT):
                for lo, hi in _chunks(DIM, HC):
                    ps = s1ps.tile([P, HC], FP, tag="s1")
                    for ki, k in enumerate(ks):
                        nc.tensor.matmul(
                            ps[:, 0:hi - lo],
                            oT_sb[:, k, i * P:(i + 1) * P],
                            wpT_sb[:, k, lo:hi],
                            start=(ki == 0),
                            stop=(ki == len(ks) - 1),
                        )
                    nc.vector.tensor_tensor(
                        out=y_acc[:, i, lo:hi], in0=ps[:, 0:hi - lo],
                        in1=bias_sb[:, lo:hi] if first else y_acc[:, i, lo:hi],
                        op=Add,
                    )
                    yield

        # ---- paired attention primitives ----
        def issue_st_half(t, j, c):
            """S^T i-chunk c for BOTH heads of pair t, key block j, into one
            [128, 1024] PSUM tile: head 2t -> cols 0:512 (partitions 0:64 of
            the qk tile), head 2t+1 -> cols 512:1024 (partitions 64:128).
            One tile = one WAR release (its exp), so the two K=64 matmuls
            stay back-to-back and run concurrently on disjoint row groups."""
            st = stps.tile([P, N], FP, tag="st")
            lo = c * HC
            for hp, off in ((0, 0), (D, HC)):
                nc.tensor.matmul(
                    st[:, off:off + HC],
                    qkT_sb[hp:hp + D, 2 * t + 1, j * P:(j + 1) * P],
                    qkT_sb[hp:hp + D, 2 * t, lo:lo + HC],
                    start=True,
                    stop=True,
                )
            return st

        def exp_half(st, c):
            ex = expp.tile([P, N], MMDT, tag=f"exc{c}",
                           bufs=12 if c == 0 else 17)
            nc.scalar.activation(out=ex, in_=st, func=Exp, scale=float(SCALE))
            return ex

        def pv_pair(t, j, ex, otp):
            """Col-tiled concurrent PV for BOTH heads of pair t: head 2t
            into PSUM partitions 0:64 (tile_position (0,0)), head 2t+1 into
            64:128 ((0,64)). One [128, 512] bank accumulates the stacked
            OT-pair - exactly the oT_sb k-tile layout."""
            for h, off, plo in ((2 * t, 0, 0), (2 * t + 1, HC, D)):
                nc.tensor.matmul(
                    otp[plo:plo + D, :],
                    v_sb[:, j, h, 0:D],
                    ex[:, off:off + HC],
                    start=(j == 0),
                    stop=(j == NT - 1),
                    skip_group_check=True,
                )

        def quad_l(j, lq, ex_prev, ex_cur):
            """Softmax denominators for the 4 heads of a pair-group via
            4-way col-tiled M=1 ones-sum matmuls (out partitions 0/32/64/96
            of one PSUM bank), accumulated over key blocks j."""
            srcs = ((ex_prev, 0), (ex_prev, HC), (ex_cur, 0), (ex_cur, HC))
            for gi, (ex, off) in enumerate(srcs):
                nc.tensor.matmul(
                    lq[32 * gi:32 * gi + 1, :],
                    ones_col,
                    ex[:, off:off + HC],
                    start=(j == 0),
                    stop=(j == NT - 1),
                    skip_group_check=True,
                    tile_position=(0, 32 * gi),
                )

        def norm_chunk(p, c, src, lq_sb, rows):
            """Normalize one 512-wide i-chunk of pair p's stacked OT:
            broadcast the two heads' denominator rows (lq_sb row rows[0] ->
            output partitions 0:64, rows[1] -> 64:128; concurrent K=1
            col-tiled matmuls into the s1 slot), fp32 reciprocal, multiply
            into oT_sb (bf16)."""
            lo = c * HC
            rA, rB = rows
            lbc = s1ps.tile([P, HC], FP, tag="s1")
            nc.tensor.matmul(lbc[0:D, :], ones128[rA:rA + 1, 0:D],
                             lq_sb[rA:rA + 1, :], start=True, stop=True,
                             tile_position=(rA, 0))
            nc.tensor.matmul(lbc[D:P, :], ones128[rB:rB + 1, 0:D],
                             lq_sb[rB:rB + 1, :], start=True, stop=True,
                             tile_position=(rB, 64))
            rb = rp.tile([P, HC], FP, tag="rb")
            nc.vector.reciprocal_approx_fast(out=rb, in_=lbc)
            nc.vector.tensor_tensor(
                out=oT_sb[:, p, lo:lo + HC], in0=src, in1=rb, op=Mult,
            )

        # ---- interleaved pair loop ----
        def filler_for_pair(t):
            def units():
                if t + 1 < NPAIR:
                    yield from gen_qk(t + 1)
                    yield from gen_v(t + 1)
                if t == 2:
                    yield from gen_proj([0, 1], first=True)
                elif t == 4:
                    yield from gen_proj([2, 3], first=False)
            # pacing: spread the units across the ~30 pulls of a pair so
            # late-pair pulls still find PE work
            skips = {2: 0.1, 4: 0.1, 5: 1.0}.get(t, 1.5)
            acc = 0.0
            for u in units():
                yield u
                acc += skips
                while acc >= 1.0:
                    yield None
                    acc -= 1.0

        def pair_attn(t, st2, filler, ctx):
            """Attention for head pair t. Even pairs: accumulate the
            stacked OT-pair and evacuate it unnormalized (denominators
            arrive with the next pair's quad). Odd pairs: run the
            quad-l c0 stream inline, then the c1 quad pass, then normalize
            both pairs of the group. Returns (next ST tiles, ctx)."""
            a, b = 2 * t, 2 * t + 1
            odd = t % 2 == 1

            def pull():
                try:
                    next(filler)
                except StopIteration:
                    pass

            exs = []
            otp = (otps.tile([P, HC], FP, tag="ot", name="otp0"),
                   otps.tile([P, HC], FP, tag="ot", name="otp1"))
            if odd:
                lq0 = otps.tile([P, HC], FP, tag="lq", name="lq0", bufs=1)
            for j in range(NT):
                st_c0, st_c1 = st2
                ex_c0 = exp_half(st_c0, 0)
                ex_c1 = exp_half(st_c1, 1)
                exs.append((ex_c0, ex_c1))
                pull()
                n0 = issue_st_half(t, j + 1, 0) if j + 1 < NT else None
                pv_pair(t, j, ex_c0, otp[0])
                if odd:
                    quad_l(j, lq0, ctx["exs"][j][0], ex_c0)
                pull()
                n1 = issue_st_half(t, j + 1, 1) if j + 1 < NT else None
                pv_pair(t, j, ex_c1, otp[1])
                pull()
                st2 = (n0, n1)
            # ---- pair end ----
            # next pair's first STs go out first so ScalarE's exp stream
            # continues immediately
            if t + 1 < NPAIR:
                nxt = (issue_st_half(t + 1, 0, 0), issue_st_half(t + 1, 0, 1))
            else:
                nxt = None
            if not odd:
                # OT leaves PSUM unnormalized; normalized next pair
                o_un = ounp.tile([P, N], FP, tag="oun")
                nc.vector.tensor_copy(out=o_un[:, 0:HC], in_=otp[0])
                pull()
                nc.vector.tensor_copy(out=o_un[:, HC:N], in_=otp[1])
                pull()
                return nxt, {"exs": exs, "o_un": o_un}
            # odd pair: finish the quad's denominators and normalize
            lq0_sb = rp.tile([P, HC], MMDT, tag="lqsb0")
            nc.vector.tensor_copy(out=lq0_sb, in_=lq0)
            pull()
            norm_chunk(t, 0, otp[0], lq0_sb, (64, 96))  # frees ot slot c0
            pull()
            lq1 = otps.tile([P, HC], FP, tag="lq", name="lq1", bufs=1)
            for j in range(NT):
                quad_l(j, lq1, ctx["exs"][j][1], exs[j][1])
                pull()
            lq1_sb = rp.tile([P, HC], MMDT, tag="lqsb1")
            nc.vector.tensor_copy(out=lq1_sb, in_=lq1)
            pull()
            norm_chunk(t, 1, otp[1], lq1_sb, (64, 96))
            o_un = ctx["o_un"]
            norm_chunk(t - 1, 0, o_un[:, 0:HC], lq0_sb, (0, 32))
            pull()
            norm_chunk(t - 1, 1, o_un[:, HC:N], lq1_sb, (0, 32))
            return nxt, {}

        # warmup: chunk-major qk(0) units; the first ST chunk goes out after
        # just two units (q-c0, k-c0) so the exp stream starts ASAP
        g0 = gen_qk(0)
        next(g0)
        next(g0)
        st_w0 = issue_st_half(0, 0, 0)
        next(g0)
        st_w1 = issue_st_half(0, 0, 1)
        for _ in g0:
            pass
        st2 = (st_w0, st_w1)
        for _ in gen_v(0):
            pass
        ctx = None
        for t in range(NPAIR):
            f = filler_for_pair(t)
            st2, ctx = pair_attn(t, st2, f, ctx)
            for _ in f:
                pass

      # -------- stage 3: last projection k-tile (5) + combine --------
      with (
            tc.tile_pool(name="s3y", bufs=4) as s3y,
            tc.tile_pool(name="s3ps", bufs=2, space="PSUM") as s3ps,
      ):
            yr = y[:].rearrange("(i p) e -> i p e", p=P)
            for i in range(NT):
                ps = s3ps.tile([P, DIM], FP, tag="y")
                for lo, hi in _chunks(DIM, HC):
                    for ki, k in enumerate((KT - 2, KT - 1)):
                        nc.tensor.matmul(
                            ps[:, lo:hi],
                            oT_sb[:, k, i * P:(i + 1) * P],
                            wpT_sb[:, k, lo:hi],
                            start=(ki == 0),
                            stop=(ki == 1),
                        )
                y_sb = s3y.tile([P, DIM], FP, tag="ysb")
                nc.vector.tensor_tensor(
                    out=y_sb, in0=ps, in1=y_acc[:, i], op=Add,
                )
                eng = nc.sync if i % 2 == 0 else nc.scalar
                eng.dma_start(out=yr[i], in_=y_sb)


def prep_inputs(x, w_qkv, w_proj, b_proj):
    x = np.asarray(x, dtype=np.float32)
    w_qkv = np.asarray(w_qkv, dtype=np.float32)
    w_proj = np.asarray(w_proj, dtype=np.float32)
    b_proj = np.asarray(b_proj, dtype=np.float32)

    w_r = w_qkv.reshape(H, D, 3, DIM)  # rows ordered (h, d, qkv)
    wq = w_r[:, :, 0, :].reshape(DIM, DIM)  # rows (h, d)
    wk = w_r[:, :, 1, :].reshape(DIM, DIM)
    wv = w_r[:, :, 2, :].reshape(DIM, DIM)
    # pair-blocked qk columns: [q_t (128) | k_t (128)] for t = 0..5
    wqk_pairs = np.empty((2 * DIM, DIM), dtype=np.float32)
    for t in range(NPAIR):
        wqk_pairs[t * 256:t * 256 + P] = wq[t * P:(t + 1) * P]
        wqk_pairs[t * 256 + P:(t + 1) * 256] = wk[t * P:(t + 1) * P]

    def flat(wT, m):  # [DIM, M] (rows = contraction) -> [P, NPAIR|1.., KT, m]
        return np.ascontiguousarray(
            wT.reshape(KT, P, wT.shape[1] // m, m).transpose(1, 2, 0, 3)
        )

    wqk_f = flat(wqk_pairs.T, 256).reshape(P, -1).astype(NP_MMDT)
    wv_f = flat(wv.T, P).reshape(P, -1).astype(NP_MMDT)
    wp_f = np.ascontiguousarray(
        w_proj.T.reshape(KT, P, DIM).transpose(1, 0, 2)
    ).reshape(P, -1).astype(NP_MMDT)
    # x: [B, N, DIM] -> per-core xT [P, KT*N] with k-tile-major layout
    xTb = x.transpose(0, 2, 1).reshape(B, KT, P, N).transpose(0, 2, 1, 3)
    xT_f = np.ascontiguousarray(xTb).reshape(B, P, KT * N).astype(NP_MMDT)
    bias = np.ascontiguousarray(b_proj.reshape(1, DIM))
    return xT_f, wqk_f, wv_f, wp_f, bias


_NC = None
last_results = None


def get_nc():
    global _NC
    if _NC is None:
        _NC = build_nc()
    return _NC


def kernel(x, w_qkv, w_proj, b_proj):
    global last_results
    from concourse.bass_utils import run_bass_kernel_spmd

    nc = get_nc()
    xT_f, wqk_f, wv_f, wp_f, bias = prep_inputs(x, w_qkv, w_proj, b_proj)
    in_maps = [
        {"xT": xT_f[c], "wqk": wqk_f, "wv": wv_f, "wp": wp_f, "bias": bias}
        for c in range(B)
    ]
    res = run_bass_kernel_spmd(nc, in_maps, core_ids=list(range(B)))
    last_results = res
    return np.stack([res.results[c]["y"] for c in range(B)], axis=0)


# revision 32
# speedup vs baseline: 1.3033x; 1.0237x over previous
"""Multi-head attention (B=8, N=1024, DIM=768, H=12) on 8 Trainium2 NeuronCores.

Sharding: pure data-parallel over the batch dimension — core c computes batch
element c end-to-end (qkv projection, softmax attention, output projection).
No collectives needed.

Numerics: matmul inputs in bf16 (x, weights, q/k, v, exp(P), softmax
denominator row for the rank-1 broadcast) with fp32 PSUM accumulation;
reciprocal, normalization and bias in fp32.

Schedule (v7): heads are processed in PAIRS (2t, 2t+1), j-synchronized.
The two K=64 QK^T matmuls of a pair land on disjoint row-groups of the PE
array (head 2t on partitions 0-63, head 2t+1 on 64-127 of the qk pair
tile) and execute CONCURRENTLY (row tiling via auto tile_position
(0,0)/(64,0)) when issued back-to-back with a common dependency release.

  Per (pair, j) the S^T block is built as TWO staggered [128, 1024] PSUM
  tiles, each holding one 512-wide i-chunk of BOTH heads (head a cols
  0:512, head b 512:1024). Each tile gets its own exp ACTIVATE, so the
  WAR release for the next j's ST chunk-pair fires as soon as ITS exp
  completes - the c0/c1 stagger means the PE never waits a full exp and
  ScalarE never gaps. PV for head 2t runs inline into OT_a; the exp
  tiles are buffered in SBUF and PV for head 2t+1 drains at the pair
  boundary (after rchain_a frees OT banks). PSUM: st(2x2) + ot(3) +
  s1(1) = 8 banks; the rchain lbc broadcasts borrow the s1 slot.

  Stage-1 (qkv projection) for pair t+1 runs as PE filler inside pair t;
  output-projection k-chains run as filler once their oT k-tiles exist
  (k0-2 during pair 3, k3-4 during pair 5, k5 after the last rchain).

DMA: all inputs are pre-arranged host-side into the exact [128, X] SBUF
layout so every transfer is one contiguous span per partition (large
descriptors); split across both HWDGE queues (sync + scalar) with the two
xT halves in parallel. y output rows alternate queues.
"""

import os
import sys

for _p in ("/opt/trn_rl_repo",):
    if _p not in sys.path:
        sys.path.insert(0, _p)

import ml_dtypes
import numpy as np

import concourse.bass as bass
import concourse.tile as tile
from concourse import bacc, mybir

B, N, DIM, H = 8, 1024, 768, 12
D = DIM // H  # 64
SCALE = D ** -0.5
P = 128
KT = DIM // P        # 6 contraction tiles over dim
NT = N // P          # 8 tiles over sequence
NPAIR = H // 2       # 6 head pairs
FP = mybir.dt.float32
BF = mybir.dt.bfloat16
MMDT = BF
NP_MMDT = ml_dtypes.bfloat16
HC = 512             # i-chunk width (PSUM bank)


def _chunks(total, size):
    return [(lo, min(lo + size, total)) for lo in range(0, total, size)]


def build_nc():
    nc = bacc.Bacc(None, target_bir_lowering=False)
    # flat [128, X] layouts, one contiguous span per partition per transfer
    xT = nc.dram_tensor("xT", [P, KT * N], MMDT, kind="ExternalInput")
    wqk = nc.dram_tensor("wqk", [P, NPAIR * KT * 256], MMDT,
                         kind="ExternalInput")
    wv = nc.dram_tensor("wv", [P, NPAIR * KT * P], MMDT, kind="ExternalInput")
    wp = nc.dram_tensor("wp", [P, KT * DIM], MMDT, kind="ExternalInput")
    bias = nc.dram_tensor("bias", [1, DIM], FP, kind="ExternalInput")
    y = nc.dram_tensor("y", [N, DIM], FP, kind="ExternalOutput")

    with tile.TileContext(nc) as tc:
        with nc.allow_low_precision(reason="bf16 matmul inputs"):
            _body(tc, xT, wqk, wv, wp, bias, y)
    nc.compile()
    return nc


def _body(tc, xT, wqk, wv, wp, bias, y):
    nc = tc.nc
    Exp = mybir.ActivationFunctionType.Exp
    Mult = mybir.AluOpType.mult
    Add = mybir.AluOpType.add

    from contextlib import ExitStack
    with tc.tile_pool(name="persist", bufs=1) as persist:
      with ExitStack() as s12:
        s1w = s12.enter_context(tc.tile_pool(name="s1w", bufs=1))
        expp = s12.enter_context(tc.tile_pool(name="expp", bufs=9))
        rp = s12.enter_context(tc.tile_pool(name="rp", bufs=2))
        ounp = s12.enter_context(tc.tile_pool(name="ounp", bufs=2))
        s1ps = s12.enter_context(tc.tile_pool(name="s1ps", bufs=1, space="PSUM"))
        stps = s12.enter_context(tc.tile_pool(name="stps", bufs=2, space="PSUM"))
        otps = s12.enter_context(tc.tile_pool(name="otps", bufs=2, space="PSUM"))

        # qkT_sb tile index 2t = q of pair t, 2t+1 = k of pair t; rows (h%2,d)
        qkT_sb = persist.tile([P, 2 * KT, N], MMDT)     # 24 KB/part
        v_sb = persist.tile([P, NT, H, D + 1], MMDT)    # 12.7 KB/part
        oT_sb = persist.tile([P, KT, N], MMDT)          # 12 KB/part
        bias_sb = persist.tile([P, DIM], FP)            # 3 KB/part
        y_acc = persist.tile([P, NT, DIM], FP)          # 24 KB/part
        ones_col = persist.tile([P, 1], MMDT)
        ones128 = persist.tile([P, P], MMDT)
        nc.vector.memset(ones_col, 1.0)
        nc.vector.memset(ones128, 1.0)

        xT_sb = s1w.tile([P, KT, N], MMDT)              # 12 KB/part
        wqkT_sb = s1w.tile([P, NPAIR, 2, KT, P], MMDT)  # 18 KB/part
        wvT_sb = s1w.tile([P, NPAIR, KT, P], MMDT)      # 9 KB/part
        wpT_sb = s1w.tile([P, KT, DIM], MMDT)           # 9 KB/part

        # ---- input DMAs: both queues in parallel, large flat descriptors
        wqk_r = wqk[:].rearrange("p (t w k m) -> p t w k m", t=NPAIR, w=2,
                                 k=KT)
        wv_r = wv[:].rearrange("p (t k m) -> p t k m", t=NPAIR, k=KT)
        xT_r = xT[:].rearrange("p (k n) -> p k n", k=KT)
        # pair-0 weights: q-half on sync, k-half on scalar (contiguous in
        # the host layout) so the first-chain gate is ~0.6MB per queue
        nc.sync.dma_start(out=wqkT_sb[:, 0, 0], in_=wqk_r[:, 0, 0])
        nc.scalar.dma_start(out=wqkT_sb[:, 0, 1], in_=wqk_r[:, 0, 1])
        for k in range(KT):
            eng = nc.sync if k % 2 == 0 else nc.scalar
            eng.dma_start(out=xT_sb[:, k], in_=xT_r[:, k])
        nc.sync.dma_start(out=wvT_sb[:, 0], in_=wv_r[:, 0])
        for t in range(1, NPAIR):
            eng = nc.sync if t % 2 else nc.scalar
            eng.dma_start(out=wqkT_sb[:, t], in_=wqk_r[:, t])
            eng.dma_start(out=wvT_sb[:, t], in_=wv_r[:, t])
        nc.scalar.dma_start(
            out=wpT_sb, in_=wp[:].rearrange("p (k m) -> p k m", k=KT))
        nc.scalar.dma_start(out=bias_sb, in_=bias[:].to_broadcast((P, DIM)))

        # ---- PE work generators (filler units of ~0.5-1.7us of matmuls) ----
        def gen_qk(t):
            """qk pair-tile t -> qkT_sb[:, 2t] (q) and [:, 2t+1] (k).
            Chunk-major unit order (q-c0, k-c0, q-c1, k-c1) so the first
            ST of a pair only needs the first two units."""
            for lo, hi in _chunks(N, HC):
                for which in range(2):
                    ps = s1ps.tile([P, HC], FP, tag="s1")
                    for k in range(KT):
                        nc.tensor.matmul(
                            ps,
                            wqkT_sb[:, t, which, k, :],
                            xT_sb[:, k, lo:hi],
                            start=(k == 0),
                            stop=(k == KT - 1),
                        )
                    nc.vector.tensor_copy(
                        out=qkT_sb[:, 2 * t + which, lo:hi], in_=ps)
                    yield

        def gen_v(t):
            """v pair-slice t -> v_sb[:, :, 2t:2t+2, 0:D]."""
            for half in range(2):
                ps = s1ps.tile([P, HC], FP, tag="s1")
                for jj in range(4):
                    j = half * 4 + jj
                    for k in range(KT):
                        nc.tensor.matmul(
                            ps[:, jj * P:(jj + 1) * P],
                            xT_sb[:, k, j * P:(j + 1) * P],
                            wvT_sb[:, t, k, :],
                            start=(k == 0),
                            stop=(k == KT - 1),
                        )
                    yield
                nc.vector.tensor_copy(
                    out=v_sb[:, half * 4:(half + 1) * 4, 2 * t:2 * t + 2, 0:D],
                    in_=ps.rearrange("p (j g d) -> p j g d", g=2, d=D),
                )

        def gen_proj(ks, first):
            """Output-projection contribution of oT k-tiles `ks`,
            SBUF-accumulated into y_acc (adds bias on the first round)."""
            for i in range(NT):
                for lo, hi in _chunks(DIM, HC):
                    ps = s1ps.tile([P, HC], FP, tag="s1")
                    for ki, k in enumerate(ks):
                        nc.tensor.matmul(
                            ps[:, 0:hi - lo],
                            oT_sb[:, k, i * P:(i + 1) * P],
                            wpT_sb[:, k, lo:hi],
                            start=(ki == 0),
                            stop=(ki == len(ks) - 1),
                        )
                    nc.vector.tensor_tensor(
                        out=y_acc[:, i, lo:hi], in0=ps[:, 0:hi - lo],
                        in1=bias_sb[:, lo:hi] if first else y_acc[:, i, lo:hi],
                        op=Add,
                    )
                    yield

        # ---- paired attention primitives ----
        def issue_st_half(t, j, c):
            """S^T i-chunk c for BOTH heads of pair t, key block j, into one
            [128, 1024] PSUM tile: head 2t -> cols 0:512 (partitions 0:64 of
            the qk tile), head 2t+1 -> cols 512:1024 (partitions 64:128).
            One tile = one WAR release (its exp), so the two K=64 matmuls
            stay back-to-back and run concurrently on disjoint row groups."""
            st = stps.tile([P, N], FP, tag="st")
            lo = c * HC
            for hp, off in ((0, 0), (D, HC)):
                nc.tensor.matmul(
                    st[:, off:off + HC],
                    qkT_sb[hp:hp + D, 2 * t + 1, j * P:(j + 1) * P],
                    qkT_sb[hp:hp + D, 2 * t, lo:lo + HC],
                    start=True,
                    stop=True,
                )
            return st

        def exp_half(st, c):
            ex = expp.tile([P, N], MMDT, tag=f"exc{c}",
                           bufs=12 if c == 0 else 17)
            nc.scalar.activation(out=ex, in_=st, func=Exp, scale=float(SCALE))
            return ex

        def pv_pair(t, j, ex, otp):
            """Col-tiled concurrent PV for BOTH heads of pair t: head 2t
            into PSUM partitions 0:64 (tile_position (0,0)), head 2t+1 into
            64:128 ((0,64)). One [128, 512] bank accumulates the stacked
            OT-pair - exactly the oT_sb k-tile layout."""
            for h, off, plo in ((2 * t, 0, 0), (2 * t + 1, HC, D)):
                nc.tensor.matmul(
                    otp[plo:plo + D, :],
                    v_sb[:, j, h, 0:D],
                    ex[:, off:off + HC],
                    start=(j == 0),
                    stop=(j == NT - 1),
                    skip_group_check=True,
                )

        def quad_l(j, lq, ex_prev, ex_cur):
            """Softmax denominators for the 4 heads of a pair-group via
            4-way col-tiled M=1 ones-sum matmuls (out partitions 0/32/64/96
            of one PSUM bank), accumulated over key blocks j."""
            srcs = ((ex_prev, 0), (ex_prev, HC), (ex_cur, 0), (ex_cur, HC))
            for gi, (ex, off) in enumerate(srcs):
                nc.tensor.matmul(
                    lq[32 * gi:32 * gi + 1, :],
                    ones_col,
                    ex[:, off:off + HC],
                    start=(j == 0),
                    stop=(j == NT - 1),
                    skip_group_check=True,
                    tile_position=(0, 32 * gi),
                )

        def norm_chunk(p, c, src, lq_sb, rows):
            """Normalize one 512-wide i-chunk of pair p's stacked OT:
            broadcast the two heads' denominator rows (lq_sb row rows[0] ->
            output partitions 0:64, rows[1] -> 64:128; concurrent K=1
            col-tiled matmuls into the s1 slot), fp32 reciprocal, multiply
            into oT_sb (bf16)."""
            lo = c * HC
            rA, rB = rows
            lbc = s1ps.tile([P, HC], FP, tag="s1")
            nc.tensor.matmul(lbc[0:D, :], ones128[rA:rA + 1, 0:D],
                             lq_sb[rA:rA + 1, :], start=True, stop=True,
                             tile_position=(rA, 0))
            nc.tensor.matmul(lbc[D:P, :], ones128[rB:rB + 1, 0:D],
                             lq_sb[rB:rB + 1, :], start=True, stop=True,
                             tile_position=(rB, 64))
            rb = rp.tile([P, HC], FP, tag="rb")
            nc.vector.reciprocal_approx_fast(out=rb, in_=lbc)
            nc.vector.tensor_tensor(
                out=oT_sb[:, p, lo:lo + HC], in0=src, in1=rb, op=Mult,
            )

        # ---- interleaved pair loop ----
        def filler_for_pair(t):
            def units():
                if t + 1 < NPAIR:
                    yield from gen_qk(t + 1)
                    yield from gen_v(t + 1)
                if t == 2:
                    yield from gen_proj([0, 1], first=True)
                elif t == 4:
                    yield from gen_proj([2, 3], first=False)
            # pacing: spread the units across the ~30 pulls of a pair so
            # late-pair pulls still find PE work
            skips = {2: 0.1, 4: 0.1, 5: 1.0}.get(t, 1.5)
            acc = 0.0
            for u in units():
                yield u
                acc += skips
                while acc >= 1.0:
                    yield None
                    acc -= 1.0

        def pair_attn(t, st2, filler, ctx):
            """Attention for head pair t. Even pairs: accumulate the
            stacked OT-pair and evacuate it unnormalized (denominators
            arrive with the next pair's quad). Odd pairs: run the
            quad-l c0 stream inline, then the c1 quad pass, then normalize
            both pairs of the group. Returns (next ST tiles, ctx)."""
            a, b = 2 * t, 2 * t + 1
            odd = t % 2 == 1

            def pull():
                try:
                    next(filler)
                except StopIteration:
                    pass

            exs = []
            otp = (otps.tile([P, HC], FP, tag="ot", name="otp0"),
                   otps.tile([P, HC], FP, tag="ot", name="otp1"))
            if odd:
                lq0 = otps.tile([P, HC], FP, tag="lq", name="lq0", bufs=1)
            for j in range(NT):
                st_c0, st_c1 = st2
                ex_c0 = exp_half(st_c0, 0)
                ex_c1 = exp_half(st_c1, 1)
                exs.append((ex_c0, ex_c1))
                pull()
                n0 = issue_st_half(t, j + 1, 0) if j + 1 < NT else None
                pv_pair(t, j, ex_c0, otp[0])
                if odd:
                    quad_l(j, lq0, ctx["exs"][j][0], ex_c0)
                pull()
                n1 = issue_st_half(t, j + 1, 1) if j + 1 < NT else None
                pv_pair(t, j, ex_c1, otp[1])
                pull()
                st2 = (n0, n1)
            # ---- pair end ----
            # next pair's first STs go out first so ScalarE's exp stream
            # continues immediately
            if t + 1 < NPAIR:
                nxt = (issue_st_half(t + 1, 0, 0), issue_st_half(t + 1, 0, 1))
            else:
                nxt = None
            if not odd:
                # OT leaves PSUM unnormalized; normalized next pair
                o_un = ounp.tile([P, N], FP, tag="oun")
                nc.vector.tensor_copy(out=o_un[:, 0:HC], in_=otp[0])
                pull()
                nc.vector.tensor_copy(out=o_un[:, HC:N], in_=otp[1])
                pull()
                return nxt, {"exs": exs, "o_un": o_un}
            # odd pair: finish the quad's denominators and normalize
            lq0_sb = rp.tile([P, HC], MMDT, tag="lqsb0")
            nc.vector.tensor_copy(out=lq0_sb, in_=lq0)
            pull()
            norm_chunk(t, 0, otp[0], lq0_sb, (64, 96))  # frees ot slot c0
            pull()
            lq1 = otps.tile([P, HC], FP, tag="lq", name="lq1", bufs=1)
            for j in range(NT):
                quad_l(j, lq1, ctx["exs"][j][1], exs[j][1])
                pull()
            lq1_sb = rp.tile([P, HC], MMDT, tag="lqsb1")
            nc.vector.tensor_copy(out=lq1_sb, in_=lq1)
            pull()
            norm_chunk(t, 1, otp[1], lq1_sb, (64, 96))
            o_un = ctx["o_un"]
            norm_chunk(t - 1, 0, o_un[:, 0:HC], lq0_sb, (0, 32))
            pull()
            norm_chunk(t - 1, 1, o_un[:, HC:N], lq1_sb, (0, 32))
            return nxt, {}

        # warmup: the two chunk-0 qk units run through the idle OT banks
        # (chains + copies pipeline; each matmul starts as its xT k-tile
        # lands), then the first ST pair goes out so exp starts ASAP
        for which in range(2):
            ps = otps.tile([P, HC], FP, tag="ot", name=f"warmqk{which}")
            for k in range(KT):
                nc.tensor.matmul(
                    ps,
                    wqkT_sb[:, 0, which, k, :],
                    xT_sb[:, k, 0:HC],
                    start=(k == 0),
                    stop=(k == KT - 1),
                )
            nc.vector.tensor_copy(out=qkT_sb[:, which, 0:HC], in_=ps)
        st_w0 = issue_st_half(0, 0, 0)
        # chunk-1 qk units (q-c1 feeds the second warm ST; k-c1 feeds j>=4)
        for which in range(2):
            ps = s1ps.tile([P, HC], FP, tag="s1", name=f"warmc1_{which}")
            for k in range(KT):
                nc.tensor.matmul(
                    ps,
                    wqkT_sb[:, 0, which, k, :],
                    xT_sb[:, k, HC:N],
                    start=(k == 0),
                    stop=(k == KT - 1),
                )
            nc.vector.tensor_copy(out=qkT_sb[:, which, HC:N], in_=ps)
            if which == 0:
                st_w1 = issue_st_half(0, 0, 1)
        st2 = (st_w0, st_w1)
        for _ in gen_v(0):
            pass
        ctx = None
        for t in range(NPAIR):
            f = filler_for_pair(t)
            st2, ctx = pair_attn(t, st2, f, ctx)
            for _ in f:
                pass

      # -------- stage 3: last projection k-tile (5) + combine --------
      with (
            tc.tile_pool(name="s3y", bufs=4) as s3y,
            tc.tile_pool(name="s3ps", bufs=2, space="PSUM") as s3ps,
      ):
            yr = y[:].rearrange("(i p) e -> i p e", p=P)
            for i in range(NT):
                ps = s3ps.tile([P, DIM], FP, tag="y")
                for lo, hi in _chunks(DIM, HC):
                    for ki, k in enumerate((KT - 2, KT - 1)):
                        nc.tensor.matmul(
                            ps[:, lo:hi],
                            oT_sb[:, k, i * P:(i + 1) * P],
                            wpT_sb[:, k, lo:hi],
                            start=(ki == 0),
                            stop=(ki == 1),
                        )
                y_sb = s3y.tile([P, DIM], FP, tag="ysb")
                nc.vector.tensor_tensor(
                    out=y_sb, in0=ps, in1=y_acc[:, i], op=Add,
                )
                eng = nc.sync if i % 2 == 0 else nc.scalar
                eng.dma_start(out=yr[i], in_=y_sb)


def prep_inputs(x, w_qkv, w_proj, b_proj):
    x = np.asarray(x, dtype=np.float32)
    w_qkv = np.asarray(w_qkv, dtype=np.float32)
    w_proj = np.asarray(w_proj, dtype=np.float32)
    b_proj = np.asarray(b_proj, dtype=np.float32)

    w_r = w_qkv.reshape(H, D, 3, DIM)  # rows ordered (h, d, qkv)
    wq = w_r[:, :, 0, :].reshape(DIM, DIM)  # rows (h, d)
    wk = w_r[:, :, 1, :].reshape(DIM, DIM)
    wv = w_r[:, :, 2, :].reshape(DIM, DIM)
    # pair-blocked qk columns: [q_t (128) | k_t (128)] for t = 0..5
    wqk_pairs = np.empty((2 * DIM, DIM), dtype=np.float32)
    for t in range(NPAIR):
        wqk_pairs[t * 256:t * 256 + P] = wq[t * P:(t + 1) * P]
        wqk_pairs[t * 256 + P:(t + 1) * 256] = wk[t * P:(t + 1) * P]

    def flat(wT, m):  # [DIM, M] (rows = contraction) -> [P, NPAIR|1.., KT, m]
        return np.ascontiguousarray(
            wT.reshape(KT, P, wT.shape[1] // m, m).transpose(1, 2, 0, 3)
        )

    # [P, pair, k, 256] -> [P, pair, which(q|k), k, 128] (contiguous halves)
    wqk_f = np.ascontiguousarray(
        flat(wqk_pairs.T, 256)
        .reshape(P, NPAIR, KT, 2, P)
        .transpose(0, 1, 3, 2, 4)
    ).reshape(P, -1).astype(NP_MMDT)
    wv_f = flat(wv.T, P).reshape(P, -1).astype(NP_MMDT)
    wp_f = np.ascontiguousarray(
        w_proj.T.reshape(KT, P, DIM).transpose(1, 0, 2)
    ).reshape(P, -1).astype(NP_MMDT)
    # x: [B, N, DIM] -> per-core xT [P, KT*N] with k-tile-major layout
    xTb = x.transpose(0, 2, 1).reshape(B, KT, P, N).transpose(0, 2, 1, 3)
    xT_f = np.ascontiguousarray(xTb).reshape(B, P, KT * N).astype(NP_MMDT)
    bias = np.ascontiguousarray(b_proj.reshape(1, DIM))
    return xT_f, wqk_f, wv_f, wp_f, bias


_NC = None
last_results = None


def get_nc():
    global _NC
    if _NC is None:
        _NC = build_nc()
    return _NC


def kernel(x, w_qkv, w_proj, b_proj):
    global last_results
    from concourse.bass_utils import run_bass_kernel_spmd

    nc = get_nc()
    xT_f, wqk_f, wv_f, wp_f, bias = prep_inputs(x, w_qkv, w_proj, b_proj)
    in_maps = [
        {"xT": xT_f[c], "wqk": wqk_f, "wv": wv_f, "wp": wp_f, "bias": bias}
        for c in range(B)
    ]
    res = run_bass_kernel_spmd(nc, in_maps, core_ids=list(range(B)))
    last_results = res
    return np.stack([res.results[c]["y"] for c in range(B)], axis=0)
